# revision 16
# baseline (speedup 1.0000x reference)
"""GQA causal attention (B=1, T=4096, D=1024, HQ=16, HKV=4, HD=64) on 8 trn2
NeuronCores via Bass/Tile.

Sharding: block-cyclic sequence-parallel for the ATTENTION q tokens. The 4096
query tokens are split into 64 blocks of 64; core i owns blocks
{i, 8+i, ..., 56+i} (512 q tokens). Every core runs the SAME program (SPMD):
for its j-th block it processes k-tiles [0, 4*(j+1)) — a core-independent
conservative causal extent — and a host-supplied per-core boundary mask zeroes
the non-causal tail, so per-core work is uniform AND balanced.

K/V are projected from a CONTIGUOUS 512-token chunk (core i handles tokens
[512i, 512(i+1))), so the AllGather output lands in global token order and the
post-collective SBUF fill is 16+8 large contiguous DMAs instead of a
fine-grained scatter. The K/V projection+gather is issued before the Q
projection so the collective overlaps Q-proj and the weight loads.

Layout strategy (avoids all on-device transposes):
  - host passes x^T; scores are computed as S^T[k, q] = (K^T)^T-tiles @ Q^T
    with k on partitions, so the softmax denominator is obtained by appending
    a ones-column to the V stationary ([V|1]) and the exp is a pure
    elementwise ACT pass PSUM->SBUF.
  - normalization is deferred: ctx^T = (sum_k e^s V) is divided by the
    rowsum (row 64 of the [V|1] matmul output) after the k-loop, via a
    reciprocal + K=1 broadcast-matmul.
  - Wq/Wo columns/rows are host-permuted so two heads stack into 128
    partitions everywhere (even-kv heads on partitions 0-63, odd-kv on
    64-127), which also lets score matmuls (contraction dim = head_dim = 64)
    run pairwise-packed in the PE array via tile_position row groups.

dtypes: x and Wq/Wk/Wv are bf16 (projection matmuls bf16 with fp32 PSUM
accumulation, FWL-eligible weight loads); score/ctx matmuls bf16; softmax
accumulation fp32 in PSUM; Wo and the output matmul run in float32r.
"""

import os
import sys

sys.path.insert(0, "/opt/trn_rl_repo")

import numpy as np
import ml_dtypes

import concourse.bass as bass
import concourse.bacc as bacc
import concourse.mybir as mybir
import concourse.tile as tile
from concourse.bass_utils import run_bass_kernel_spmd

# ---------------------------------------------------------------- constants
B, T, D = 1, 4096, 1024
HQ, HKV, HD = 16, 4, 64
G = HQ // HKV          # 4 q heads per kv head
NC = 8                 # cores
QB = 64                # q block size
NBLK = T // QB         # 64 blocks total
BPC = NBLK // NC       # 8 blocks per core
LQ = QB * BPC          # 512 local q tokens per core
DT = D // 128          # 8 contraction tiles over D
NKT = T // 128         # 32 k-tiles
F32 = mybir.dt.float32
F32R = mybir.dt.float32r
BF16 = mybir.dt.bfloat16
BF16NP = ml_dtypes.bfloat16

# head pairing: pair tile m holds (LO[m] on partitions 0-63, HI[m] on 64-127).
LO = [0, 1, 2, 3, 8, 9, 10, 11]
HI = [4, 5, 6, 7, 12, 13, 14, 15]


def _local_cols(i):
    """Global token indices owned by core i, in local order."""
    return np.concatenate(
        [QB * (NC * j + i) + np.arange(QB) for j in range(BPC)]
    )


def _band_mask(i):
    """[4, 128, 64] multiplicative causal mask for the last k-quartet of any
    block: valid iff 128*kt2 + p <= 64*i + f."""
    kt2 = np.arange(4)[:, None, None]
    p = np.arange(128)[None, :, None]
    f = np.arange(64)[None, None, :]
    return (128 * kt2 + p <= 64 * i + f).astype(BF16NP)


NE = 3  # early chunks (K/V projected locally on every core, no gather wait)


# ---------------------------------------------------------------- program
def build_nc():
    nc = bacc.Bacc(None)
    xo_d = nc.declare_dram_parameter("xT_own", [D, LQ], BF16, isOutput=False)
    xc_d = nc.declare_dram_parameter("xT_chunk", [D, LQ], BF16, isOutput=False)
    xe_d = nc.declare_dram_parameter("xT_early", [D, NE * LQ], BF16, isOutput=False)
    wq_d = nc.declare_dram_parameter("Wq_perm", [D, HQ * HD], BF16, isOutput=False)
    wk_d = nc.declare_dram_parameter("Wk_n", [D, HKV * HD], BF16, isOutput=False)
    wv_d = nc.declare_dram_parameter("Wv_n", [D, HKV * HD], BF16, isOutput=False)
    wo_d = nc.declare_dram_parameter("Wo_perm", [HQ * HD, D], BF16, isOutput=False)
    bm_d = nc.declare_dram_parameter("bmask", [4, 128, QB], BF16, isOutput=False)
    on_d = nc.declare_dram_parameter("ones_c", [1, HD], F32R, isOutput=False)
    out_d = nc.declare_dram_parameter("out_loc", [LQ, D], F32, isOutput=True)

    with tile.TileContext(nc) as tc:
        _emit(nc, tc, xo_d, xc_d, xe_d, wq_d, wk_d, wv_d, wo_d, bm_d, on_d, out_d)
    nc.finalize()
    return nc


def _emit(nc, tc, xo_d, xc_d, xe_d, wq_d, wk_d, wv_d, wo_d, bm_d, on_d, out_d):
    from contextlib import ExitStack

    es = ExitStack()
    with es:
        sb = es.enter_context(tc.tile_pool(name="sb", bufs=2))
        sb3 = es.enter_context(tc.tile_pool(name="sb3", bufs=6))
        res = es.enter_context(tc.tile_pool(name="res", bufs=1))
        ps2 = es.enter_context(tc.tile_pool(name="ps2", bufs=2, space="PSUM"))
        dram = es.enter_context(tc.tile_pool(name="dramkv", bufs=1, space="DRAM"))

        # ---------------- resident tensors. ONLY the KV-proj inputs (2 MB)
        # load up front — everything else is sequenced behind the ko/vo
        # bounce writes on the same Sync queue, so the HBM link belongs to
        # the collective critical path first.
        xc = res.tile([128, DT, LQ], BF16, tag="xc")          # x^T contiguous
        nc.sync.dma_start(xc[:], xc_d.rearrange("(dt p) t -> p dt t", p=128))
        wk = res.tile([128, DT, HKV * HD], BF16, tag="wk")
        nc.sync.dma_start(wk[:], wk_d.rearrange("(dt p) h -> p dt h", p=128))
        wv = res.tile([128, DT, HKV * HD], BF16, tag="wv")
        nc.sync.dma_start(wv[:], wv_d.rearrange("(dt p) h -> p dt h", p=128))
        bm = res.tile([128, 4, QB], BF16, tag="bm")           # band masks
        nc.scalar.dma_start(bm[:], bm_d.rearrange("k p f -> p k f"))
        xe = res.tile([128, NE, DT, LQ], BF16, tag="xe")      # x^T first chunks
        xo = res.tile([128, DT, LQ], BF16, tag="xo")          # x^T own cols

        kt_sb = [res.tile([128, T], BF16, tag=f"kt{h2}", name=f"kt{h2}") for h2 in range(2)]
        v_sb = res.tile([128, NKT, HKV, HD + 1], BF16, tag="v")  # [V | 1]
        qg_sb = [
            res.tile([128, G, LQ], BF16, tag=f"qg{h2}", name=f"qg{h2}")
            for h2 in range(2)
        ]
        ctx_sb = res.tile([128, 8, LQ], BF16, tag="ctx")      # normalized ctx^T
        ones_sb = res.tile([1, HD], F32R, tag="ones")
        nc.scalar.dma_start(ones_sb[:], on_d[:])
        nc.vector.memset(v_sb[:, :, :, HD : HD + 1], 1.0)

        # ---------------- P0: K^T projection of the contiguous chunk and its
        # AllGather doorbell fire FIRST; the V projection + gather follow.
        ko_sb = res.tile([128, 2, LQ], BF16, tag="ko_sb")
        vo_sb = res.tile([128, 4, HKV * HD], BF16, tag="vo_sb")
        for h2 in range(2):  # K^T chunk: [128(2 heads), LQ] per kv-pair
            psk = ps2.tile([128, LQ], F32, tag="scores", name="psk")
            for d in range(DT):
                nc.tensor.matmul(
                    psk[:], wk[:, d, 128 * h2 : 128 * (h2 + 1)], xc[:, d, :],
                    start=(d == 0), stop=(d == DT - 1),
                )
            nc.vector.tensor_copy(ko_sb[:, h2, :], psk[:])
        ko_d = dram.tile([2 * 128, LQ], BF16, name="ko_d")
        nc.sync.dma_start(ko_d.rearrange("(h2 p) t -> p h2 t", p=128), ko_sb[:])
        kg_d = dram.tile([NC * 2 * 128, LQ], BF16, name="kg_d", addr_space="Shared")
        vg_d = dram.tile([NC * LQ, HKV * HD], BF16, name="vg_d", addr_space="Shared")
        nc.gpsimd.collective_compute(
            "AllGather", mybir.AluOpType.bypass,
            replica_groups=[list(range(NC))],
            ins=[ko_d[:]], outs=[kg_d[:]],
        )
        for tq in range(4):  # V chunk natural: [128 t, 256]
            psv = ps2.tile([128, HKV * HD], F32, tag="scores", name="psv")
            for d in range(DT):
                nc.tensor.matmul(
                    psv[:], xc[:, d, 128 * tq : 128 * (tq + 1)], wv[:, d, :],
                    start=(d == 0), stop=(d == DT - 1),
                )
            nc.vector.tensor_copy(vo_sb[:, tq, :], psv[:])
        vo_d = dram.tile([LQ, HKV * HD], BF16, name="vo_d")
        nc.sync.dma_start(vo_d.rearrange("(tq p) h -> p tq h", p=128), vo_sb[:])
        nc.gpsimd.collective_compute(
            "AllGather", mybir.AluOpType.bypass,
            replica_groups=[list(range(NC))],
            ins=[vo_d[:]], outs=[vg_d[:]],
        )

        # Remaining loads, sequenced behind the bounce writes on the Sync
        # FIFO: they start only once the collective doorbell has fired, and
        # overlap its ~30us execution latency.
        nc.sync.dma_start(xo[:], xo_d.rearrange("(dt p) q -> p dt q", p=128))
        wqt = res.tile([128, DT, HQ * HD], BF16, tag="wqt", name="wqt")
        nc.sync.dma_start(wqt[:], wq_d.rearrange("(dt p) h -> p dt h", p=128))
        nc.sync.dma_start(
            xe[:], xe_d.rearrange("(dt p) (e t) -> p e dt t", p=128, e=NE)
        )
        out_sb = res.tile([128, 4, D], F32, tag="osb")
        wot = res.tile([128, 8, D], BF16, tag="wot", name="wot")
        nc.sync.dma_start(wot[:], wo_d.rearrange("(m p) dcol -> p m dcol", p=128))

        # ---------------- P1: Q^T projection (overlaps the AllGather),
        # scaled by HD^-0.5
        for m in range(8):
            psq = ps2.tile([128, LQ], F32, tag="pacc", name=f"psq{m}")
            for d in range(DT):
                nc.tensor.matmul(
                    psq[:],
                    wqt[:, d, 128 * m : 128 * (m + 1)],
                    xo[:, d, :],
                    start=(d == 0),
                    stop=(d == DT - 1),
                )
            # cast to bf16 with the 1/sqrt(HD) score scale folded in
            nc.vector.tensor_scalar_mul(
                qg_sb[m // 4][:, m % 4, :], psq[:], float(HD) ** -0.5
            )

        # ---------------- P1b: K^T/V for the first NE chunks, projected
        # locally from x_early (no collective wait) straight into SBUF.
        for e in range(NE):
            for h2 in range(2):
                psk = ps2.tile([128, LQ], F32, tag="scores", name=f"pske{e}{h2}")
                for d in range(DT):
                    nc.tensor.matmul(
                        psk[:], wk[:, d, 128 * h2 : 128 * (h2 + 1)],
                        xe[:, e, d, :],
                        start=(d == 0), stop=(d == DT - 1),
                    )
                nc.vector.tensor_copy(
                    kt_sb[h2][:, LQ * e : LQ * (e + 1)], psk[:]
                )
            for tq in range(4):
                psv = ps2.tile([128, HKV * HD], F32, tag="scores", name=f"psve{e}{tq}")
                for d in range(DT):
                    nc.tensor.matmul(
                        psv[:], xe[:, e, d, 128 * tq : 128 * (tq + 1)],
                        wv[:, d, :],
                        start=(d == 0), stop=(d == DT - 1),
                    )
                nc.vector.tensor_copy(
                    v_sb[:, 4 * e + tq, :, 0:HD],
                    psv.rearrange("p (h e2) -> p h e2", h=HKV),
                )

        # ---------------- P2: SBUF fill from the gathered buffers for the
        # remaining chunks. The AG output is already in global token order
        # (contiguous chunks), so these are large contiguous DMAs.
        kgv = kg_d.rearrange("(c h2 p) t -> p c h2 t", h2=2, p=128)
        vgv = vg_d.rearrange("(kt p) (hv e) -> p kt hv e", p=128, hv=HKV)
        for c in range(NE, NC):
            for h2 in range(2):
                nc.sync.dma_start(
                    kt_sb[h2][:, LQ * c : LQ * (c + 1)], kgv[:, c, h2, :]
                )
            for kt in range(4 * c, 4 * (c + 1)):
                nc.sync.dma_start(v_sb[:, kt, :, 0:HD], vgv[:, kt])

        # ---------------- P3: attention over blocks, with the output
        # projection (f32r) interleaved per block-pair on the PE's slack.
        for j in range(BPC):
            nkp = 2 * (j + 1)  # k-tile pairs this block
            ctx_ps = [
                ps2.tile([HD + 1, 8 * QB], F32, tag="ctx", name=f"ctxps{h2}_{j}")
                for h2 in range(2)
            ]
            for kp in range(nkp):
                for h2 in range(2):
                    s_ps = ps2.tile([128, 2, 2, G, QB], F32, tag="scores")
                    qsl = slice(QB * j, QB * (j + 1))
                    for kt2 in range(2):
                        kt = 2 * kp + kt2
                        ksl = slice(128 * kt, 128 * (kt + 1))
                        for hs in range(2):
                            # one matmul covers all G q-heads of this kv head
                            nc.tensor.matmul(
                                s_ps[:, hs, kt2, :, :],
                                kt_sb[h2][64 * hs : 64 * hs + 64, ksl],
                                qg_sb[h2][64 * hs : 64 * hs + 64, :, qsl],
                                start=True, stop=True,
                                tile_position=(64 * hs, 0),
                            )
                    pt = sb3.tile([128, 2, 2, G, QB], BF16, tag="pt")
                    nc.scalar.activation(
                        pt[:], s_ps[:], mybir.ActivationFunctionType.Exp
                    )
                    if kp >= 2 * j:  # boundary quartet: apply causal mask
                        par = kp - 2 * j
                        msk = bm[:, 2 * par : 2 * par + 2, None, :].to_broadcast(
                            (128, 2, G, QB)
                        )
                        for hs in range(2):
                            nc.vector.tensor_mul(pt[:, hs], pt[:, hs], msk)
                    for kt2 in range(2):
                        kt = 2 * kp + kt2
                        for hs in range(2):
                            kv = 2 * h2 + hs
                            # start=True only on the very first matmul into this
                            # psum tile: start marks the whole 2KB zero-region
                            # pending-zero, so each slot's first write overwrites
                            # (correct) and later writes accumulate.
                            nc.tensor.matmul(
                                ctx_ps[h2][:, 256 * hs : 256 * (hs + 1)],
                                v_sb[:, kt, kv, :],
                                pt[:, hs, kt2, :, :],
                                start=(kp == 0 and kt2 == 0 and hs == 0),
                                stop=(kp == nkp - 1 and kt2 == 1),
                                skip_group_check=True,
                            )
            # ---- normalize: ctx / rowsum (row HD of ctx_ps)
            rs = sb.tile([1, 2, 8 * QB], F32R, tag="rs")
            for h2 in range(2):
                nc.vector.tensor_copy(rs[0:1, h2, :], ctx_ps[h2][HD : HD + 1, :])
            hi_st = sb.tile([64, 8, QB], BF16, tag="hist")
            for h2 in range(2):
                # broadcast rowsum over 64 partitions FIRST, then take the
                # reciprocal on 64 lanes
                bc = ps2.tile([HD, 8 * QB], F32, tag="pacc", name="bc")
                nc.tensor.matmul(
                    bc[:], ones_sb[:], rs[0:1, h2, :],
                    start=True, stop=True,
                )
                bcs = sb.tile([HD, 8 * QB], F32, tag="bcs")
                nc.vector.reciprocal_approx_fast(out=bcs[:], in_=bc[:])
                for hs in range(2):
                    for mq in range(4):
                        s = 4 * hs + mq
                        m = 4 * h2 + mq
                        ssl = slice(QB * s, QB * (s + 1))
                        if hs == 0:
                            nc.vector.tensor_mul(
                                ctx_sb[0:64, m, QB * j : QB * (j + 1)],
                                ctx_ps[h2][0:HD, ssl],
                                bcs[:, ssl],
                            )
                        else:
                            nc.vector.tensor_mul(
                                hi_st[:, m, :], ctx_ps[h2][0:HD, ssl], bcs[:, ssl]
                            )
            # partition-shift the odd-kv heads to partitions 64-127 (DMA)
            nc.sync.dma_start(
                ctx_sb[64:128, :, QB * j : QB * (j + 1)], hi_st[:]
            )
            # ---- output projection for the completed block pair
            if j % 2 == 1:
                tt = j // 2
                for dc in range(2):
                    pso = ps2.tile([128, 512], F32, tag="pacc", name=f"pso{tt}_{dc}")
                    for m in range(8):
                        nc.tensor.matmul(
                            pso[:],
                            ctx_sb[:, m, 128 * tt : 128 * (tt + 1)],
                            wot[:, m, 512 * dc : 512 * (dc + 1)],
                            start=(m == 0),
                            stop=(m == 7),
                        )
                    nc.vector.tensor_copy(
                        out_sb[:, tt, 512 * dc : 512 * (dc + 1)], pso[:]
                    )
                nc.sync.dma_start(
                    out_d.rearrange("(tt p) dcol -> p tt dcol", p=128)[:, tt],
                    out_sb[:, tt],
                )


def _install_ntff_hook():
    """Provide antenv.axon_hooks (absent from this image's antenv) so that
    run_bass_kernel_spmd(trace=True) can NTFF-profile via libaxon_pjrt."""
    import sys as _sys
    import types as _types

    if "antenv.axon_hooks" not in _sys.modules:
        import antenv as _antenv

        mod = _types.ModuleType("antenv.axon_hooks")
        mod._HOOK = None

        def _set(h, _m=mod):
            _m._HOOK = h

        def _get(_m=mod):
            return _m._HOOK

        mod.set_axon_ntff_profile_hook = _set
        mod.get_axon_ntff_profile_hook = _get
        _sys.modules["antenv.axon_hooks"] = mod
        _antenv.axon_hooks = mod
    mod = _sys.modules["antenv.axon_hooks"]
    if mod.get_axon_ntff_profile_hook() is None:
        import trn_agent_boot.trn_boot as _tb

        hook = _tb._ntff_profile_via_ctypes("/opt/axon/libaxon_pjrt.so")
        mod.set_axon_ntff_profile_hook(hook)
    # artifact upload needs a bucket this sandbox doesn't have
    from concourse import bass_utils as _bu

    _bu.upload_artifacts = lambda tmpdir: f"local://{tmpdir}"


# ---------------------------------------------------------------- host side
_NC_CACHE = None


def _get_nc():
    global _NC_CACHE
    if _NC_CACHE is None:
        _NC_CACHE = build_nc()
    return _NC_CACHE


def _prep_in_maps(x, Wq, Wk, Wv, Wo):
    xT = np.ascontiguousarray(x[0].T).astype(np.float32)          # [D, T]
    xT_bf = xT.astype(BF16NP)
    wq_perm = np.empty_like(Wq)
    wo_perm = np.empty_like(Wo)
    for m in range(8):
        wq_perm[:, 128 * m : 128 * m + 64] = Wq[:, 64 * LO[m] : 64 * LO[m] + 64]
        wq_perm[:, 128 * m + 64 : 128 * m + 128] = Wq[:, 64 * HI[m] : 64 * HI[m] + 64]
        wo_perm[128 * m : 128 * m + 64, :] = Wo[64 * LO[m] : 64 * LO[m] + 64, :]
        wo_perm[128 * m + 64 : 128 * m + 128, :] = Wo[64 * HI[m] : 64 * HI[m] + 64, :]
    wq_bf = wq_perm.astype(BF16NP)
    wk_bf = Wk.astype(BF16NP)
    wv_bf = Wv.astype(BF16NP)
    maps = []
    for i in range(NC):
        cols = _local_cols(i)
        m = {
            "xT_own": np.ascontiguousarray(xT_bf[:, cols]),
            "xT_chunk": np.ascontiguousarray(xT_bf[:, LQ * i : LQ * (i + 1)]),
            "xT_early": np.ascontiguousarray(xT_bf[:, 0 : NE * LQ]),
            "Wq_perm": wq_bf,
            "Wk_n": wk_bf,
            "Wv_n": wv_bf,
            "Wo_perm": np.ascontiguousarray(wo_perm.astype(BF16NP)),
            "bmask": _band_mask(i),
            "ones_c": np.ones((1, HD), np.float32),
        }
        maps.append(m)
    return maps


def kernel(x, Wq, Wk, Wv, Wo):
    nc = _get_nc()
    maps = _prep_in_maps(
        np.asarray(x, np.float32),
        np.asarray(Wq, np.float32),
        np.asarray(Wk, np.float32),
        np.asarray(Wv, np.float32),
        np.asarray(Wo, np.float32),
    )
    trace = bool(int(os.environ.get("KERNEL_TRACE", "0")))
    if trace:
        try:
            _install_ntff_hook()
        except Exception as e:  # profiling is best-effort
            print(f"ntff hook install failed: {e}")
    r = run_bass_kernel_spmd(nc, maps, list(range(NC)), trace=trace)
    out = np.empty((B, T, D), np.float32)
    for i in range(NC):
        out[0, _local_cols(i), :] = r.results[i]["out_loc"]
    if trace:
        kernel.last_exec_time_ns = r.exec_time_ns
        kernel.last_results = r
    return out


if __name__ == "__main__":
    pass
